# Initial kernel scaffold
#
"""Slot-attention corrector kernel for Trainium2 (8 NeuronCores, data-parallel).

Layout strategy per core (8 examples):
  - host sends x (natural, bf16) for LN stats and xT (transposed, bf16) for matmuls
  - LN folded into k/v projections via rank-1 mu-correction + per-row rstd scale
  - k stored transposed  kT [d=128, n=4096] bf16 (lhsT for dots)
  - v stored natural     v  [n, d] bf16          (rhs for updates)
  - dots^T [n, s] layout -> softmax over slots is a free-axis reduction
  - GRU/MLP on [128, 128] batched slot state, fp32 throughout
"""

import numpy as np
import ml_dtypes
import sys

sys.path.insert(0, "/opt/trn_rl_repo")

NUM_SLOTS, SLOT_DIM, FEAT_DIM, HID_DIM = 16, 128, 512, 512
EPS_LN = 1e-3
SCALE = FEAT_DIM ** -0.5
B, N = 64, 4096
NCORES = 8
BEX = B // NCORES          # 8 examples per core
NBLK = N // 128            # 32 n-blocks per example
NCH = N // 512             # 8 n-chunks of 512
FCH = FEAT_DIM // 128      # 4 f-chunks

_CACHE = {}


def _build(num_iters: int, general_bias: bool, reps: int = 1):
    import concourse.bass as bass
    import concourse.bacc as bacc
    import concourse.tile as tile
    from concourse import mybir

    f32 = mybir.dt.float32
    bf16 = mybir.dt.bfloat16
    AF = mybir.ActivationFunctionType
    AX = mybir.AxisListType

    nc = bacc.Bacc('TRN2', target_bir_lowering=False, debug=False, enable_asserts=False, num_devices=NCORES)

    # ---------------- dram I/O ----------------
    x_d = nc.dram_tensor("x", [BEX, N, FEAT_DIM], bf16, kind="ExternalInput")
    xT_d = nc.dram_tensor("xT", [BEX, FEAT_DIM, N], bf16, kind="ExternalInput")
    slots_d = nc.dram_tensor("slots0", [128, SLOT_DIM], f32, kind="ExternalInput")
    wkv_d = nc.dram_tensor("wkv", [FEAT_DIM, 256], bf16, kind="ExternalInput")
    ckv_d = nc.dram_tensor("ckv", [1, 256], bf16, kind="ExternalInput")
    bkv_d = nc.dram_tensor("bkv", [1, 256], f32, kind="ExternalInput")  # [bk'|bv']
    bk_col_d = nc.dram_tensor("bk_col", [128, 1], f32, kind="ExternalInput")
    wq_d = nc.dram_tensor("wq", [SLOT_DIM, SLOT_DIM], f32, kind="ExternalInput")
    bqs_col_d = nc.dram_tensor("bqs_col", [128, 1], f32, kind="ExternalInput")
    wihT_d = nc.dram_tensor("wihT", [SLOT_DIM, 3 * SLOT_DIM], f32, kind="ExternalInput")
    whhT_d = nc.dram_tensor("whhT", [SLOT_DIM, 3 * SLOT_DIM], f32, kind="ExternalInput")
    bih_d = nc.dram_tensor("bih_row", [1, 3 * SLOT_DIM], f32, kind="ExternalInput")
    bhh_d = nc.dram_tensor("bhh_row", [1, 3 * SLOT_DIM], f32, kind="ExternalInput")
    w1_d = nc.dram_tensor("w1", [SLOT_DIM, HID_DIM], f32, kind="ExternalInput")
    b1c_d = nc.dram_tensor("b1_cols", [128, 4], f32, kind="ExternalInput")
    w2_d = nc.dram_tensor("w2", [HID_DIM, SLOT_DIM], f32, kind="ExternalInput")
    b2_d = nc.dram_tensor("b2_row", [1, SLOT_DIM], f32, kind="ExternalInput")
    ones_f_d = nc.dram_tensor("ones_f", [128, 128], f32, kind="ExternalInput")
    ones_b_d = nc.dram_tensor("ones_b", [128, 128], bf16, kind="ExternalInput")
    ident_d = nc.dram_tensor("ident", [128, 128], f32, kind="ExternalInput")
    out_d = nc.dram_tensor("out", [128, SLOT_DIM], f32, kind="ExternalOutput")

    with tile.TileContext(nc) as tc:
        with (
            tc.tile_pool(name="kv", bufs=1) as kvp,          # resident k/v (16MB)
            tc.tile_pool(name="consts", bufs=1) as cp,
            tc.tile_pool(name="dram", bufs=2, space="DRAM") as dp,
        ):
            # ---- resident k/v ----
            kT = [kvp.tile([128, N], bf16, tag=f"kT{e}", name=f"kT{e}") for e in range(BEX)]
            vN = [kvp.tile([128, NBLK * 128], bf16, tag=f"v{e}", name=f"v{e}") for e in range(BEX)]

            # ---- constants ----
            wkv_sb = cp.tile([FEAT_DIM // 4, 4, 256], bf16)  # [128f, fch, 256]
            for j in range(FCH):
                nc.sync.dma_start(out=wkv_sb[:, j, :], in_=wkv_d[j * 128:(j + 1) * 128, :])
            ckv_sb = cp.tile([1, 256], bf16)
            nc.sync.dma_start(out=ckv_sb, in_=ckv_d[:, :])
            wq_sb = cp.tile([128, 128], f32)
            nc.sync.dma_start(out=wq_sb, in_=wq_d[:, :])
            bqs_sb = cp.tile([128, 1], f32)
            nc.sync.dma_start(out=bqs_sb, in_=bqs_col_d[:, :])
            wih_sb = cp.tile([128, 384], f32)
            nc.sync.dma_start(out=wih_sb, in_=wihT_d[:, :])
            whh_sb = cp.tile([128, 384], f32)
            nc.sync.dma_start(out=whh_sb, in_=whhT_d[:, :])
            bih_sb = cp.tile([1, 384], f32)
            nc.sync.dma_start(out=bih_sb, in_=bih_d[:, :])
            bhh_sb = cp.tile([1, 384], f32)
            nc.sync.dma_start(out=bhh_sb, in_=bhh_d[:, :])
            w1_sb = cp.tile([128, 512], f32)
            nc.sync.dma_start(out=w1_sb, in_=w1_d[:, :])
            b1c_sb = cp.tile([128, 4], f32)
            nc.sync.dma_start(out=b1c_sb, in_=b1c_d[:, :])
            w2_sb = cp.tile([128, 4, 128], f32)  # [128h, chunk, 128d]
            for j in range(4):
                nc.sync.dma_start(out=w2_sb[:, j, :], in_=w2_d[j * 128:(j + 1) * 128, :])
            b2_sb = cp.tile([1, 128], f32)
            nc.sync.dma_start(out=b2_sb, in_=b2_d[:, :])
            ones_f = cp.tile([128, 128], f32)
            nc.sync.dma_start(out=ones_f, in_=ones_f_d[:, :])
            ones_b = cp.tile([128, 128], bf16)
            nc.sync.dma_start(out=ones_b, in_=ones_b_d[:, :])
            ident = cp.tile([128, 128], f32)
            nc.sync.dma_start(out=ident, in_=ident_d[:, :])
            eps_col = cp.tile([128, 1], f32)
            nc.vector.memset(eps_col, EPS_LN)
            neg1_col = cp.tile([128, 1], f32)
            nc.vector.memset(neg1_col, -1.0)
            if general_bias:
                bk_col = cp.tile([128, 1], f32)
                nc.sync.dma_start(out=bk_col, in_=bk_col_d[:, :])
                bv_bc = cp.tile([128, 128], f32)
                nc.gpsimd.dma_start(
                    out=bv_bc,
                    in_=bass.AP(tensor=bkv_d, offset=128, ap=[[0, 128], [1, 128]]),
                )


            for _rep in range(reps):
                slots = cp.tile([128, 128], f32, tag="slots_state")
                nc.sync.dma_start(out=slots, in_=slots_d[:, :])
                # ================= PHASE 1: stats + k/v production =================
                with (
                    tc.tile_pool(name="p1x", bufs=3) as p1x,
                    tc.tile_pool(name="p1xt", bufs=1) as p1xt,
                    tc.tile_pool(name="p1s", bufs=2) as p1s,
                    tc.tile_pool(name="p1ps", bufs=2, space="PSUM") as p1ps,
                    tc.tile_pool(name="p1pv", bufs=2, space="PSUM") as p1pv,
                    tc.tile_pool(name="p1pt", bufs=2, space="PSUM") as p1pt,
                    tc.tile_pool(name="p1row", bufs=1) as p1row,
                ):
                    for e in range(BEX):
                        # ---- stats over natural x ----
                        aggr = p1s.tile([128, NBLK, 2], f32, tag="aggr")
                        for t in range(NBLK):
                            xt = p1x.tile([128, FEAT_DIM], bf16, tag="xt")
                            nc.gpsimd.dma_start(out=xt, in_=x_d[e, t * 128:(t + 1) * 128, :])
                            st = p1x.tile([128, 6], f32, tag="st")
                            nc.vector.bn_stats(out=st, in_=xt)
                            nc.vector.bn_aggr(out=aggr[:, t, :], in_=st)
                        std_nat = p1s.tile([128, NBLK], f32, tag="std_nat")
                        nc.scalar.activation(std_nat, aggr[:, :, 1], AF.Sqrt, bias=eps_col)
                        rstd_nat = p1s.tile([128, NBLK], f32, tag="rstd_nat")
                        nc.vector.reciprocal(rstd_nat, std_nat)
                        nmu_nat = p1s.tile([128, NBLK], f32, tag="nmu_nat")
                        nc.scalar.activation(nmu_nat, aggr[:, :, 0], AF.Copy, scale=neg1_col)
                        # transpose stats -> rows [1, 4096] via dram bounce
                        rowbuf = {}
                        drbuf = {}
                        for name, src in (("rstd", rstd_nat), ("nmu", nmu_nat)):
                            ps = p1pt.tile([NBLK, 128], f32, tag="statT")
                            nc.tensor.transpose(ps, src, ident)
                            sb = p1row.tile([NBLK, 128], bf16, tag="statT_sb")
                            nc.scalar.activation(sb, ps, AF.Copy)
                            dr = dp.tile([NBLK, 128], bf16, tag="bounce")
                            nc.sync.dma_start(out=dr, in_=sb)
                            row = p1row.tile([1, N], bf16, tag=f"{name}_row")
                            nc.gpsimd.dma_start(
                                out=row,
                                in_=bass.AP(tensor=dr.tensor, offset=dr.offset, ap=[[0, 1], [1, N]]),
                            )
                            rowbuf[name] = row
                            drbuf[name] = dr
                        rstd_bc = p1row.tile([128, N], bf16, tag="rstd_bc")
                        nc.gpsimd.dma_start(
                            out=rstd_bc,
                            in_=bass.AP(tensor=drbuf["rstd"].tensor, offset=drbuf["rstd"].offset,
                                        ap=[[0, 128], [1, N]]),
                        )

                        # ---- xT tiles for this example ----
                        xTt = [p1xt.tile([128, N], bf16, tag=f"xT{j}", name=f"xTt{j}") for j in range(FCH)]
                        for j in range(FCH):
                            nc.sync.dma_start(out=xTt[j], in_=xT_d[e, j * 128:(j + 1) * 128, :])

                        # ---- kT production (Wk stationary-ish, N=512 chunks) ----
                        for c in range(NCH):
                            ps = p1ps.tile([128, 512], f32, tag="kTps")
                            for j in range(FCH):
                                nc.tensor.matmul(
                                    ps, wkv_sb[:, j, 0:128], xTt[j][:, c * 512:(c + 1) * 512],
                                    start=(j == 0), stop=False,
                                )
                            nc.tensor.matmul(
                                ps, ckv_sb[:, 0:128], rowbuf["nmu"][:, c * 512:(c + 1) * 512],
                                start=False, stop=True,
                            )
                            nc.vector.tensor_mul(kT[e][:, c * 512:(c + 1) * 512], ps,
                                                 rstd_bc[:, c * 512:(c + 1) * 512])
                        if general_bias:
                            nc.scalar.activation(kT[e], kT[e], AF.Identity, bias=bk_col)

                        # ---- v production (natural) ----
                        for t in range(NBLK):
                            ps = p1pv.tile([128, 128], f32, tag="vps")
                            for j in range(FCH):
                                nc.tensor.matmul(
                                    ps, xTt[j][:, t * 128:(t + 1) * 128], wkv_sb[:, j, 128:256],
                                    start=(j == 0), stop=False,
                                )
                            nc.tensor.matmul(
                                ps, rowbuf["nmu"][:, t * 128:(t + 1) * 128], ckv_sb[:, 128:256],
                                start=False, stop=True,
                            )
                            if general_bias:
                                nc.vector.tensor_add(ps, ps, bv_bc)
                            nc.scalar.activation(
                                vN[e][:, t * 128:(t + 1) * 128], ps, AF.Copy,
                                scale=rstd_nat[:, t:t + 1],
                            )

                # ================= PHASE 2: iterations =================
                with (
                    tc.tile_pool(name="itw", bufs=2) as itw,
                    tc.tile_pool(name="attn", bufs=2) as atp,
                    tc.tile_pool(name="pdots", bufs=2, space="PSUM") as pdots,
                    tc.tile_pool(name="pupd", bufs=2, space="PSUM") as pupd,
                    tc.tile_pool(name="pz", bufs=1, space="PSUM") as pz,
                    tc.tile_pool(name="pt", bufs=1, space="PSUM") as pt,
                    tc.tile_pool(name="pmm", bufs=2, space="PSUM") as pmm,
                ):
                    def layernorm_t(src, tag):
                        """LN over free dim of [128,128] fp32 src -> (ln_sb, lnT_sb)."""
                        st = itw.tile([128, 6], f32, tag=f"{tag}_st")
                        nc.vector.bn_stats(out=st, in_=src)
                        mv = itw.tile([128, 2], f32, tag=f"{tag}_mv")
                        nc.vector.bn_aggr(out=mv, in_=st)
                        std = itw.tile([128, 1], f32, tag=f"{tag}_std")
                        nc.scalar.activation(std, mv[:, 1:2], AF.Sqrt, bias=eps_col)
                        rstd = itw.tile([128, 1], f32, tag=f"{tag}_rstd")
                        nc.vector.reciprocal(rstd, std)
                        nmu = itw.tile([128, 1], f32, tag=f"{tag}_nmu")
                        nc.scalar.activation(nmu, mv[:, 0:1], AF.Copy, scale=neg1_col)
                        nmr = itw.tile([128, 1], f32, tag=f"{tag}_nmr")
                        nc.vector.tensor_mul(nmr, nmu, rstd)
                        ln = itw.tile([128, 128], f32, tag=f"{tag}_ln")
                        nc.scalar.activation(ln, src, AF.Identity, scale=rstd, bias=nmr)
                        ps = pt.tile([128, 128], f32, tag="transp")
                        nc.tensor.transpose(ps, ln, ident)
                        lnT = itw.tile([128, 128], f32, tag=f"{tag}_lnT")
                        nc.scalar.activation(lnT, ps, AF.Copy)
                        return ln, lnT

                    for it in range(num_iters):
                        # ---- q ----
                        _, lnT = layernorm_t(slots, "q")
                        qps = pmm.tile([128, 128], f32, tag="mmout")
                        nc.tensor.matmul(qps, wq_sb, lnT)
                        qT = itw.tile([128, 128], bf16, tag="qT")
                        nc.scalar.activation(qT, qps, AF.Identity, bias=bqs_sb)

                        updT = itw.tile([128, 128], f32, tag="updT")
                        zps = pz.tile([16, 8], f32, tag="zps")
                        for e in range(BEX):
                            dps = pdots.tile([128, 512], f32, tag="dots")
                            for t in range(NBLK):
                                nc.tensor.matmul(
                                    dps[:, t * 16:(t + 1) * 16],
                                    kT[e][:, t * 128:(t + 1) * 128],
                                    qT[:, e * 16:(e + 1) * 16],
                                )
                            E = atp.tile([128, 512], f32, tag="E")
                            nc.scalar.activation(E, dps, AF.Exp)
                            den = atp.tile([128, 32], f32, tag="den")
                            nc.vector.reduce_sum(
                                den, bass.AP(tensor=E.tensor, offset=E.offset,
                                             ap=[E.ap[0], [16, 32], [1, 16]]),
                                axis=AX.X,
                            )
                            rden = atp.tile([128, 32], f32, tag="rden")
                            nc.vector.reciprocal(rden, den)
                            attn = atp.tile([128, 512], bf16, tag="attn")
                            nc.vector.tensor_mul(
                                bass.AP(tensor=attn.tensor, offset=attn.offset,
                                        ap=[attn.ap[0], [16, 32], [1, 16]]),
                                bass.AP(tensor=E.tensor, offset=E.offset,
                                        ap=[E.ap[0], [16, 32], [1, 16]]),
                                bass.AP(tensor=rden.tensor, offset=rden.offset,
                                        ap=[rden.ap[0], [1, 32], [0, 16]]),
                            )
                            ups = pupd.tile([16, 128], f32, tag="upd")
                            for t in range(NBLK):
                                nc.tensor.matmul(
                                    ups, attn[:, t * 16:(t + 1) * 16],
                                    vN[e][:, t * 128:(t + 1) * 128],
                                    start=(t == 0), stop=(t == NBLK - 1),
                                )
                                nc.tensor.matmul(
                                    zps[:, e:e + 1], attn[:, t * 16:(t + 1) * 16],
                                    ones_b[:, 0:1],
                                    start=(t == 0), stop=(t == NBLK - 1),
                                )
                            rz = atp.tile([16, 1], f32, tag="rz")
                            nc.vector.reciprocal(rz, zps[:, e:e + 1])
                            usb = atp.tile([16, 128], f32, tag="usb")
                            nc.scalar.activation(usb, ups, AF.Copy, scale=rz)
                            tp = pt.tile([128, 128], f32, tag="transp")
                            nc.tensor.transpose(tp[:, 0:16], usb, ident[0:16, 0:16])
                            nc.scalar.activation(updT[:, e * 16:(e + 1) * 16], tp[:, 0:16], AF.Copy)

                        # ---- GRU ----
                        gips = pmm.tile([128, 384], f32, tag="mmout")
                        nc.tensor.matmul(gips, updT, wih_sb, start=True, stop=False)
                        nc.tensor.matmul(gips, ones_f[0:1, :], bih_sb, start=False, stop=True)
                        tp = pt.tile([128, 128], f32, tag="transp")
                        nc.tensor.transpose(tp, slots, ident)
                        slotsT = itw.tile([128, 128], f32, tag="slotsT")
                        nc.scalar.activation(slotsT, tp, AF.Copy)
                        ghps = pmm.tile([128, 384], f32, tag="mmout")
                        nc.tensor.matmul(ghps, slotsT, whh_sb, start=True, stop=False)
                        nc.tensor.matmul(ghps, ones_f[0:1, :], bhh_sb, start=False, stop=True)
                        gh_sb = itw.tile([128, 384], f32, tag="gh_sb")
                        nc.scalar.activation(gh_sb, ghps, AF.Copy)
                        rzin = itw.tile([128, 256], f32, tag="rzin")
                        nc.vector.tensor_add(rzin, gips[:, 0:256], gh_sb[:, 0:256])
                        rzg = itw.tile([128, 256], f32, tag="rzg")
                        nc.scalar.activation(rzg, rzin, AF.Sigmoid)
                        hnr = itw.tile([128, 128], f32, tag="hnr")
                        nc.vector.tensor_mul(hnr, rzg[:, 0:128], gh_sb[:, 256:384])
                        nin = itw.tile([128, 128], f32, tag="nin")
                        nc.vector.tensor_add(nin, gips[:, 256:384], hnr)
                        ng = itw.tile([128, 128], f32, tag="ng")
                        nc.scalar.activation(ng, nin, AF.Tanh)
                        hmn = itw.tile([128, 128], f32, tag="hmn")
                        nc.vector.tensor_sub(hmn, slots, ng)
                        zh = itw.tile([128, 128], f32, tag="zh")
                        nc.vector.tensor_mul(zh, rzg[:, 128:256], hmn)
                        hgru = itw.tile([128, 128], f32, tag="hgru")
                        nc.vector.tensor_add(hgru, ng, zh)

                        # ---- MLP ----
                        _, lnmT = layernorm_t(hgru, "m")
                        h1r = itw.tile([128, 4, 128], f32, tag="h1r")
                        for j in range(4):
                            hp = pmm.tile([128, 128], f32, tag="mmout")
                            nc.tensor.matmul(hp, w1_sb[:, j * 128:(j + 1) * 128], lnmT)
                            nc.scalar.activation(h1r[:, j, :], hp, AF.Relu, bias=b1c_sb[:, j:j + 1])
                        h2ps = pmm.tile([128, 128], f32, tag="mmout")
                        for j in range(4):
                            nc.tensor.matmul(h2ps, h1r[:, j, :], w2_sb[:, j, :],
                                             start=(j == 0), stop=False)
                        nc.tensor.matmul(h2ps, ones_f[0:1, :], b2_sb, start=False, stop=True)
                        new_slots = cp.tile([128, 128], f32, tag="slots_state")
                        nc.vector.tensor_add(new_slots, h2ps, hgru)
                        slots = new_slots

                    nc.sync.dma_start(out=out_d[:, :], in_=slots)

    nc.finalize()
    return nc


def _prep_host(inputs):
    f = np.float32
    g_in = inputs["ln_in_g"].astype(f)
    b_in = inputs["ln_in_b"].astype(f)
    Wk = inputs["Wk"].astype(f)
    Wv = inputs["Wv"].astype(f)
    Wkp = g_in[:, None] * Wk
    Wvp = g_in[:, None] * Wv
    wkv = np.concatenate([Wkp, Wvp], axis=1)                      # [512, 256]
    ckv = wkv.sum(axis=0, keepdims=True)                          # [1, 256]
    bk = b_in @ Wk + inputs["bk"].astype(f)
    bv = b_in @ Wv + inputs["bv"].astype(f)
    bkv = np.concatenate([bk, bv])[None, :]                       # [1, 256]
    g_s = inputs["ln_slot_g"].astype(f)
    b_s = inputs["ln_slot_b"].astype(f)
    Wq = inputs["Wq"].astype(f)
    wqp = g_s[:, None] * Wq
    bqs = (b_s @ Wq + inputs["bq"].astype(f)) * np.float32(SCALE)
    g_m = inputs["ln_mlp_g"].astype(f)
    b_m = inputs["ln_mlp_b"].astype(f)
    W1 = inputs["W1"].astype(f)
    w1p = g_m[:, None] * W1
    b1p = b_m @ W1 + inputs["b1"].astype(f)                       # [512]
    bf = ml_dtypes.bfloat16
    consts = dict(
        wkv=wkv.astype(bf),
        ckv=ckv.astype(bf),
        bkv=bkv.astype(f),
        bk_col=bk[:, None].astype(f),
        wq=(wqp * np.float32(SCALE)).astype(f),
        bqs_col=bqs[:, None].astype(f),
        wihT=np.ascontiguousarray(inputs["W_ih"].astype(f).T),
        whhT=np.ascontiguousarray(inputs["W_hh"].astype(f).T),
        bih_row=inputs["b_ih"].astype(f)[None, :],
        bhh_row=inputs["b_hh"].astype(f)[None, :],
        w1=w1p.astype(f),
        b1_cols=np.ascontiguousarray(b1p.reshape(4, 128).T).astype(f),
        w2=inputs["W2"].astype(f),
        b2_row=inputs["b2"].astype(f)[None, :],
        ones_f=np.ones((128, 128), f),
        ones_b=np.ones((128, 128), bf),
        ident=np.eye(128, dtype=f),
    )
    general_bias = not (
        np.all(b_in == 0) and np.all(inputs["bk"] == 0) and np.all(inputs["bv"] == 0)
    )
    return consts, general_bias


def kernel(**inputs) -> np.ndarray:
    from concourse.bass_utils import run_bass_kernel_spmd

    is_first = int(np.asarray(inputs["is_first"]))
    num_iters = 3 if is_first else 2
    consts, general_bias = _prep_host(inputs)

    key = (num_iters, general_bias)
    if key not in _CACHE:
        _CACHE[key] = _build(num_iters, general_bias)
    nc = _CACHE[key]

    bf = ml_dtypes.bfloat16
    x = inputs["image_features"].astype(np.float32)
    xb = x.astype(bf)                                             # [64, 4096, 512]
    xTb = np.ascontiguousarray(x.transpose(0, 2, 1)).astype(bf)   # [64, 512, 4096]
    slots = inputs["slots"].astype(np.float32)                    # [64, 16, 128]

    in_maps = []
    for c in range(NCORES):
        sl = slice(c * BEX, (c + 1) * BEX)
        m = dict(consts)
        m["x"] = xb[sl]
        m["xT"] = xTb[sl]
        m["slots0"] = slots[sl].reshape(128, SLOT_DIM)
        in_maps.append(m)

    res = run_bass_kernel_spmd(nc, in_maps, list(range(NCORES)))
    out = np.stack([res.results[c]["out"] for c in range(NCORES)])  # [8, 128, 128]
    return out.reshape(B, NUM_SLOTS, SLOT_DIM)


if __name__ == "__main__":
    import reference
    inp = reference.setup_inputs()
    inp = {k: np.asarray(v) for k, v in inp.items()}
    got = kernel(**inp)
    exp = np.asarray(reference.reference(**reference.setup_inputs()))
    err = np.linalg.norm(got - exp) / np.linalg.norm(exp)
    print("Relative error:", err)



# revision 1
# speedup vs baseline: 1.4586x; 1.4586x over previous
"""Slot-attention corrector kernel for Trainium2 (8 NeuronCores, data-parallel).

Layout strategy per core (8 examples):
  - host sends x (natural, bf16) for LN stats and xT (transposed, bf16) for matmuls
  - LN folded into k/v projections via rank-1 mu-correction + per-row rstd scale
  - k stored transposed  kT [d=128, n=4096] bf16 (lhsT for dots)
  - v stored natural     v  [n, d] bf16          (rhs for updates)
  - dots^T [n, s] layout -> softmax over slots is a free-axis reduction
  - GRU/MLP on [128, 128] batched slot state, fp32 throughout
"""

import numpy as np
import ml_dtypes
import sys

sys.path.insert(0, "/opt/trn_rl_repo")

NUM_SLOTS, SLOT_DIM, FEAT_DIM, HID_DIM = 16, 128, 512, 512
EPS_LN = 1e-3
SCALE = FEAT_DIM ** -0.5
B, N = 64, 4096
NCORES = 8
BEX = B // NCORES          # 8 examples per core
NBLK = N // 128            # 32 n-blocks per example
NCH = N // 512             # 8 n-chunks of 512
FCH = FEAT_DIM // 128      # 4 f-chunks

_CACHE = {}


def _build(num_iters: int, general_bias: bool, reps: int = 1):
    import concourse.bass as bass
    import concourse.bacc as bacc
    import concourse.tile as tile
    from concourse import mybir

    f32 = mybir.dt.float32
    bf16 = mybir.dt.bfloat16
    AF = mybir.ActivationFunctionType
    AX = mybir.AxisListType

    nc = bacc.Bacc('TRN2', target_bir_lowering=False, debug=False, enable_asserts=False, num_devices=NCORES)

    # ---------------- dram I/O ----------------
    x_d = nc.dram_tensor("x", [BEX, N, FEAT_DIM], bf16, kind="ExternalInput")
    xT_d = nc.dram_tensor("xT", [BEX, FEAT_DIM, N], bf16, kind="ExternalInput")
    slots_d = nc.dram_tensor("slots0", [128, SLOT_DIM], f32, kind="ExternalInput")
    wkv_d = nc.dram_tensor("wkv", [FEAT_DIM, 256], bf16, kind="ExternalInput")
    ckv_d = nc.dram_tensor("ckv", [1, 256], bf16, kind="ExternalInput")
    bkv_d = nc.dram_tensor("bkv", [1, 256], f32, kind="ExternalInput")  # [bk'|bv']
    bk_col_d = nc.dram_tensor("bk_col", [128, 1], f32, kind="ExternalInput")
    wq_d = nc.dram_tensor("wq", [SLOT_DIM, SLOT_DIM], f32, kind="ExternalInput")
    bqs_col_d = nc.dram_tensor("bqs_col", [128, 1], f32, kind="ExternalInput")
    wihT_d = nc.dram_tensor("wihT", [SLOT_DIM, 3 * SLOT_DIM], f32, kind="ExternalInput")
    whhT_d = nc.dram_tensor("whhT", [SLOT_DIM, 3 * SLOT_DIM], f32, kind="ExternalInput")
    bih_d = nc.dram_tensor("bih_row", [1, 3 * SLOT_DIM], f32, kind="ExternalInput")
    bhh_d = nc.dram_tensor("bhh_row", [1, 3 * SLOT_DIM], f32, kind="ExternalInput")
    w1_d = nc.dram_tensor("w1", [SLOT_DIM, HID_DIM], f32, kind="ExternalInput")
    b1c_d = nc.dram_tensor("b1_cols", [128, 4], f32, kind="ExternalInput")
    w2_d = nc.dram_tensor("w2", [HID_DIM, SLOT_DIM], f32, kind="ExternalInput")
    b2_d = nc.dram_tensor("b2_row", [1, SLOT_DIM], f32, kind="ExternalInput")
    ones_f_d = nc.dram_tensor("ones_f", [128, 128], f32, kind="ExternalInput")
    ones_b_d = nc.dram_tensor("ones_b", [128, 128], bf16, kind="ExternalInput")
    ident_d = nc.dram_tensor("ident", [128, 128], f32, kind="ExternalInput")
    out_d = nc.dram_tensor("out", [128, SLOT_DIM], f32, kind="ExternalOutput")

    with tile.TileContext(nc) as tc:
        with (
            tc.tile_pool(name="kv", bufs=1) as kvp,          # resident k/v (16MB)
            tc.tile_pool(name="consts", bufs=1) as cp,
            tc.tile_pool(name="dram", bufs=2, space="DRAM") as dp,
        ):
            # ---- resident k/v ----
            kT = [kvp.tile([128, N], bf16, tag=f"kT{e}", name=f"kT{e}") for e in range(BEX)]
            vN = [kvp.tile([128, NBLK * 128], bf16, tag=f"v{e}", name=f"v{e}") for e in range(BEX)]

            # ---- constants ----
            wkv_sb = cp.tile([FEAT_DIM // 4, 4, 256], bf16)  # [128f, fch, 256]
            for j in range(FCH):
                nc.sync.dma_start(out=wkv_sb[:, j, :], in_=wkv_d[j * 128:(j + 1) * 128, :])
            ckv_sb = cp.tile([1, 256], bf16)
            nc.sync.dma_start(out=ckv_sb, in_=ckv_d[:, :])
            wq_sb = cp.tile([128, 128], f32)
            nc.sync.dma_start(out=wq_sb, in_=wq_d[:, :])
            bqs_sb = cp.tile([128, 1], f32)
            nc.sync.dma_start(out=bqs_sb, in_=bqs_col_d[:, :])
            wih_sb = cp.tile([128, 384], f32)
            nc.sync.dma_start(out=wih_sb, in_=wihT_d[:, :])
            whh_sb = cp.tile([128, 384], f32)
            nc.sync.dma_start(out=whh_sb, in_=whhT_d[:, :])
            bih_sb = cp.tile([1, 384], f32)
            nc.sync.dma_start(out=bih_sb, in_=bih_d[:, :])
            bhh_sb = cp.tile([1, 384], f32)
            nc.sync.dma_start(out=bhh_sb, in_=bhh_d[:, :])
            w1_sb = cp.tile([128, 512], f32)
            nc.sync.dma_start(out=w1_sb, in_=w1_d[:, :])
            b1c_sb = cp.tile([128, 4], f32)
            nc.sync.dma_start(out=b1c_sb, in_=b1c_d[:, :])
            w2_sb = cp.tile([128, 4, 128], f32)  # [128h, chunk, 128d]
            for j in range(4):
                nc.sync.dma_start(out=w2_sb[:, j, :], in_=w2_d[j * 128:(j + 1) * 128, :])
            b2_sb = cp.tile([1, 128], f32)
            nc.sync.dma_start(out=b2_sb, in_=b2_d[:, :])
            ones_f = cp.tile([128, 128], f32)
            nc.sync.dma_start(out=ones_f, in_=ones_f_d[:, :])
            ones_b = cp.tile([128, 128], bf16)
            nc.sync.dma_start(out=ones_b, in_=ones_b_d[:, :])
            ident = cp.tile([128, 128], f32)
            nc.sync.dma_start(out=ident, in_=ident_d[:, :])
            eps_col = cp.tile([128, 1], f32)
            nc.vector.memset(eps_col, EPS_LN)
            neg1_col = cp.tile([128, 1], f32)
            nc.vector.memset(neg1_col, -1.0)
            if general_bias:
                bk_col = cp.tile([128, 1], f32)
                nc.sync.dma_start(out=bk_col, in_=bk_col_d[:, :])
                bv_bc = cp.tile([128, 128], f32)
                nc.gpsimd.dma_start(
                    out=bv_bc,
                    in_=bass.AP(tensor=bkv_d, offset=128, ap=[[0, 128], [1, 128]]),
                )


            for _rep in range(reps):
                slots = cp.tile([128, 128], f32, tag="slots_state")
                nc.sync.dma_start(out=slots, in_=slots_d[:, :])
                # ================= PHASE 1: stats + k/v production =================
                with (
                    tc.tile_pool(name="p1x", bufs=3) as p1x,
                    tc.tile_pool(name="p1xt", bufs=1) as p1xt,
                    tc.tile_pool(name="p1s", bufs=2) as p1s,
                    tc.tile_pool(name="p1ps", bufs=2, space="PSUM") as p1ps,
                    tc.tile_pool(name="p1pv", bufs=2, space="PSUM") as p1pv,
                    tc.tile_pool(name="p1pt", bufs=2, space="PSUM") as p1pt,
                    tc.tile_pool(name="p1row", bufs=1) as p1row,
                ):
                    for e in range(BEX):
                        # ---- stats over natural x ----
                        aggr = p1s.tile([128, NBLK, 2], f32, tag="aggr")
                        for t in range(NBLK):
                            xt = p1x.tile([128, FEAT_DIM], bf16, tag="xt")
                            nc.gpsimd.dma_start(out=xt, in_=x_d[e, t * 128:(t + 1) * 128, :])
                            st = p1x.tile([128, 6], f32, tag="st")
                            nc.vector.bn_stats(out=st, in_=xt)
                            nc.vector.bn_aggr(out=aggr[:, t, :], in_=st)
                        std_nat = p1s.tile([128, NBLK], f32, tag="std_nat")
                        nc.scalar.activation(std_nat, aggr[:, :, 1], AF.Sqrt, bias=eps_col)
                        rstd_nat = p1s.tile([128, NBLK], f32, tag="rstd_nat")
                        nc.vector.reciprocal(rstd_nat, std_nat)
                        nmu_nat = p1s.tile([128, NBLK], f32, tag="nmu_nat")
                        nc.scalar.activation(nmu_nat, aggr[:, :, 0], AF.Copy, scale=neg1_col)
                        # transpose stats -> rows [1, 4096] via dram bounce
                        rowbuf = {}
                        drbuf = {}
                        for name, src in (("rstd", rstd_nat), ("nmu", nmu_nat)):
                            ps = p1pt.tile([NBLK, 128], f32, tag="statT")
                            nc.tensor.transpose(ps, src, ident)
                            sb = p1row.tile([NBLK, 128], bf16, tag="statT_sb")
                            nc.scalar.activation(sb, ps, AF.Copy)
                            dr = dp.tile([NBLK, 128], bf16, tag="bounce")
                            nc.sync.dma_start(out=dr, in_=sb)
                            row = p1row.tile([1, N], bf16, tag=f"{name}_row")
                            nc.gpsimd.dma_start(
                                out=row,
                                in_=bass.AP(tensor=dr.tensor, offset=dr.offset, ap=[[0, 1], [1, N]]),
                            )
                            rowbuf[name] = row
                            drbuf[name] = dr
                        rstd_bc = p1row.tile([128, N], bf16, tag="rstd_bc")
                        nc.gpsimd.dma_start(
                            out=rstd_bc,
                            in_=bass.AP(tensor=drbuf["rstd"].tensor, offset=drbuf["rstd"].offset,
                                        ap=[[0, 128], [1, N]]),
                        )

                        # ---- xT tiles for this example ----
                        xTt = [p1xt.tile([128, N], bf16, tag=f"xT{j}", name=f"xTt{j}") for j in range(FCH)]
                        for j in range(FCH):
                            nc.sync.dma_start(out=xTt[j], in_=xT_d[e, j * 128:(j + 1) * 128, :])

                        # ---- kT production (Wk stationary-ish, N=512 chunks) ----
                        for c in range(NCH):
                            ps = p1ps.tile([128, 512], f32, tag="kTps")
                            for j in range(FCH):
                                nc.tensor.matmul(
                                    ps, wkv_sb[:, j, 0:128], xTt[j][:, c * 512:(c + 1) * 512],
                                    start=(j == 0), stop=False,
                                )
                            nc.tensor.matmul(
                                ps, ckv_sb[:, 0:128], rowbuf["nmu"][:, c * 512:(c + 1) * 512],
                                start=False, stop=True,
                            )
                            nc.vector.tensor_mul(kT[e][:, c * 512:(c + 1) * 512], ps,
                                                 rstd_bc[:, c * 512:(c + 1) * 512])
                        if general_bias:
                            nc.scalar.activation(kT[e], kT[e], AF.Identity, bias=bk_col)

                        # ---- v production (natural) ----
                        for t in range(NBLK):
                            ps = p1pv.tile([128, 128], f32, tag="vps")
                            for j in range(FCH):
                                nc.tensor.matmul(
                                    ps, xTt[j][:, t * 128:(t + 1) * 128], wkv_sb[:, j, 128:256],
                                    start=(j == 0), stop=False,
                                )
                            nc.tensor.matmul(
                                ps, rowbuf["nmu"][:, t * 128:(t + 1) * 128], ckv_sb[:, 128:256],
                                start=False, stop=True,
                            )
                            if general_bias:
                                nc.vector.tensor_add(ps, ps, bv_bc)
                            nc.scalar.activation(
                                vN[e][:, t * 128:(t + 1) * 128], ps, AF.Copy,
                                scale=rstd_nat[:, t:t + 1],
                            )

                # ================= PHASE 2: iterations =================
                with (
                    tc.tile_pool(name="itw", bufs=2) as itw,
                    tc.tile_pool(name="attn", bufs=2) as atp,
                    tc.tile_pool(name="pdots", bufs=2, space="PSUM") as pdots,
                    tc.tile_pool(name="pupd", bufs=2, space="PSUM") as pupd,
                    tc.tile_pool(name="pz", bufs=1, space="PSUM") as pz,
                    tc.tile_pool(name="pt", bufs=1, space="PSUM") as pt,
                    tc.tile_pool(name="pmm", bufs=2, space="PSUM") as pmm,
                ):
                    def layernorm_t(src, tag):
                        """LN over free dim of [128,128] fp32 src -> (ln_sb, lnT_sb)."""
                        st = itw.tile([128, 6], f32, tag=f"{tag}_st")
                        nc.vector.bn_stats(out=st, in_=src)
                        mv = itw.tile([128, 2], f32, tag=f"{tag}_mv")
                        nc.vector.bn_aggr(out=mv, in_=st)
                        std = itw.tile([128, 1], f32, tag=f"{tag}_std")
                        nc.scalar.activation(std, mv[:, 1:2], AF.Sqrt, bias=eps_col)
                        rstd = itw.tile([128, 1], f32, tag=f"{tag}_rstd")
                        nc.vector.reciprocal(rstd, std)
                        nmu = itw.tile([128, 1], f32, tag=f"{tag}_nmu")
                        nc.scalar.activation(nmu, mv[:, 0:1], AF.Copy, scale=neg1_col)
                        nmr = itw.tile([128, 1], f32, tag=f"{tag}_nmr")
                        nc.vector.tensor_mul(nmr, nmu, rstd)
                        ln = itw.tile([128, 128], f32, tag=f"{tag}_ln")
                        nc.scalar.activation(ln, src, AF.Identity, scale=rstd, bias=nmr)
                        ps = pt.tile([128, 128], f32, tag="transp")
                        nc.tensor.transpose(ps, ln, ident)
                        lnT = itw.tile([128, 128], f32, tag=f"{tag}_lnT")
                        nc.scalar.activation(lnT, ps, AF.Copy)
                        return ln, lnT

                    for it in range(num_iters):
                        # ---- q ----
                        _, lnT = layernorm_t(slots, "q")
                        qps = pmm.tile([128, 128], f32, tag="mmout")
                        nc.tensor.matmul(qps, wq_sb, lnT)
                        qT = itw.tile([128, 128], bf16, tag="qT")
                        nc.scalar.activation(qT, qps, AF.Identity, bias=bqs_sb)

                        updT = itw.tile([128, 128], f32, tag="updT")
                        zps = pz.tile([16, 8], f32, tag="zps")
                        for e in range(BEX):
                            dps = pdots.tile([128, 512], f32, tag="dots")
                            for t in range(NBLK):
                                nc.tensor.matmul(
                                    dps[:, t * 16:(t + 1) * 16],
                                    kT[e][:, t * 128:(t + 1) * 128],
                                    qT[:, e * 16:(e + 1) * 16],
                                )
                            E = atp.tile([128, 512], f32, tag="E")
                            nc.scalar.activation(E, dps, AF.Exp)
                            den = atp.tile([128, 32], f32, tag="den")
                            nc.vector.reduce_sum(
                                den, bass.AP(tensor=E.tensor, offset=E.offset,
                                             ap=[E.ap[0], [16, 32], [1, 16]]),
                                axis=AX.X,
                            )
                            rden = atp.tile([128, 32], f32, tag="rden")
                            nc.vector.reciprocal(rden, den)
                            attn = atp.tile([128, 512], bf16, tag="attn")
                            nc.vector.tensor_mul(
                                bass.AP(tensor=attn.tensor, offset=attn.offset,
                                        ap=[attn.ap[0], [16, 32], [1, 16]]),
                                bass.AP(tensor=E.tensor, offset=E.offset,
                                        ap=[E.ap[0], [16, 32], [1, 16]]),
                                bass.AP(tensor=rden.tensor, offset=rden.offset,
                                        ap=[rden.ap[0], [1, 32], [0, 16]]),
                            )
                            ups = pupd.tile([16, 128], f32, tag="upd")
                            for t in range(NBLK):
                                nc.tensor.matmul(
                                    ups, attn[:, t * 16:(t + 1) * 16],
                                    vN[e][:, t * 128:(t + 1) * 128],
                                    start=(t == 0), stop=(t == NBLK - 1),
                                )
                                nc.tensor.matmul(
                                    zps[:, e:e + 1], attn[:, t * 16:(t + 1) * 16],
                                    ones_b[:, 0:1],
                                    start=(t == 0), stop=(t == NBLK - 1),
                                )
                            rz = atp.tile([16, 1], f32, tag="rz")
                            nc.vector.reciprocal(rz, zps[:, e:e + 1])
                            usb = atp.tile([16, 128], f32, tag="usb")
                            nc.scalar.activation(usb, ups, AF.Copy, scale=rz)
                            tp = pt.tile([128, 128], f32, tag="transp")
                            nc.tensor.transpose(tp[:, 0:16], usb, ident[0:16, 0:16])
                            nc.scalar.activation(updT[:, e * 16:(e + 1) * 16], tp[:, 0:16], AF.Copy)

                        # ---- GRU ----
                        gips = pmm.tile([128, 384], f32, tag="mmout")
                        nc.tensor.matmul(gips, updT, wih_sb, start=True, stop=False)
                        nc.tensor.matmul(gips, ones_f[0:1, :], bih_sb, start=False, stop=True)
                        tp = pt.tile([128, 128], f32, tag="transp")
                        nc.tensor.transpose(tp, slots, ident)
                        slotsT = itw.tile([128, 128], f32, tag="slotsT")
                        nc.scalar.activation(slotsT, tp, AF.Copy)
                        ghps = pmm.tile([128, 384], f32, tag="mmout")
                        nc.tensor.matmul(ghps, slotsT, whh_sb, start=True, stop=False)
                        nc.tensor.matmul(ghps, ones_f[0:1, :], bhh_sb, start=False, stop=True)
                        gh_sb = itw.tile([128, 384], f32, tag="gh_sb")
                        nc.scalar.activation(gh_sb, ghps, AF.Copy)
                        rzin = itw.tile([128, 256], f32, tag="rzin")
                        nc.vector.tensor_add(rzin, gips[:, 0:256], gh_sb[:, 0:256])
                        rzg = itw.tile([128, 256], f32, tag="rzg")
                        nc.scalar.activation(rzg, rzin, AF.Sigmoid)
                        hnr = itw.tile([128, 128], f32, tag="hnr")
                        nc.vector.tensor_mul(hnr, rzg[:, 0:128], gh_sb[:, 256:384])
                        nin = itw.tile([128, 128], f32, tag="nin")
                        nc.vector.tensor_add(nin, gips[:, 256:384], hnr)
                        ng = itw.tile([128, 128], f32, tag="ng")
                        nc.scalar.activation(ng, nin, AF.Tanh)
                        hmn = itw.tile([128, 128], f32, tag="hmn")
                        nc.vector.tensor_sub(hmn, slots, ng)
                        zh = itw.tile([128, 128], f32, tag="zh")
                        nc.vector.tensor_mul(zh, rzg[:, 128:256], hmn)
                        hgru = itw.tile([128, 128], f32, tag="hgru")
                        nc.vector.tensor_add(hgru, ng, zh)

                        # ---- MLP ----
                        _, lnmT = layernorm_t(hgru, "m")
                        h1r = itw.tile([128, 4, 128], f32, tag="h1r")
                        for j in range(4):
                            hp = pmm.tile([128, 128], f32, tag="mmout")
                            nc.tensor.matmul(hp, w1_sb[:, j * 128:(j + 1) * 128], lnmT)
                            nc.scalar.activation(h1r[:, j, :], hp, AF.Relu, bias=b1c_sb[:, j:j + 1])
                        h2ps = pmm.tile([128, 128], f32, tag="mmout")
                        for j in range(4):
                            nc.tensor.matmul(h2ps, h1r[:, j, :], w2_sb[:, j, :],
                                             start=(j == 0), stop=False)
                        nc.tensor.matmul(h2ps, ones_f[0:1, :], b2_sb, start=False, stop=True)
                        new_slots = cp.tile([128, 128], f32, tag="slots_state")
                        nc.vector.tensor_add(new_slots, h2ps, hgru)
                        slots = new_slots

                    nc.sync.dma_start(out=out_d[:, :], in_=slots)

    nc.finalize()
    return nc


def _prep_host(inputs):
    f = np.float32
    g_in = inputs["ln_in_g"].astype(f)
    b_in = inputs["ln_in_b"].astype(f)
    Wk = inputs["Wk"].astype(f)
    Wv = inputs["Wv"].astype(f)
    Wkp = g_in[:, None] * Wk
    Wvp = g_in[:, None] * Wv
    wkv = np.concatenate([Wkp, Wvp], axis=1)                      # [512, 256]
    ckv = wkv.sum(axis=0, keepdims=True)                          # [1, 256]
    bk = b_in @ Wk + inputs["bk"].astype(f)
    bv = b_in @ Wv + inputs["bv"].astype(f)
    bkv = np.concatenate([bk, bv])[None, :]                       # [1, 256]
    g_s = inputs["ln_slot_g"].astype(f)
    b_s = inputs["ln_slot_b"].astype(f)
    Wq = inputs["Wq"].astype(f)
    wqp = g_s[:, None] * Wq
    bqs = (b_s @ Wq + inputs["bq"].astype(f)) * np.float32(SCALE)
    g_m = inputs["ln_mlp_g"].astype(f)
    b_m = inputs["ln_mlp_b"].astype(f)
    W1 = inputs["W1"].astype(f)
    w1p = g_m[:, None] * W1
    b1p = b_m @ W1 + inputs["b1"].astype(f)                       # [512]
    bf = ml_dtypes.bfloat16
    consts = dict(
        wkv=wkv.astype(bf),
        ckv=ckv.astype(bf),
        bkv=bkv.astype(f),
        bk_col=bk[:, None].astype(f),
        wq=(wqp * np.float32(SCALE)).astype(f),
        bqs_col=bqs[:, None].astype(f),
        wihT=np.ascontiguousarray(inputs["W_ih"].astype(f).T),
        whhT=np.ascontiguousarray(inputs["W_hh"].astype(f).T),
        bih_row=inputs["b_ih"].astype(f)[None, :],
        bhh_row=inputs["b_hh"].astype(f)[None, :],
        w1=w1p.astype(f),
        b1_cols=np.ascontiguousarray(b1p.reshape(4, 128).T).astype(f),
        w2=inputs["W2"].astype(f),
        b2_row=inputs["b2"].astype(f)[None, :],
        ones_f=np.ones((128, 128), f),
        ones_b=np.ones((128, 128), bf),
        ident=np.eye(128, dtype=f),
    )
    general_bias = not (
        np.all(b_in == 0) and np.all(inputs["bk"] == 0) and np.all(inputs["bv"] == 0)
    )
    return consts, general_bias


def kernel(**inputs) -> np.ndarray:
    from concourse.bass_utils import run_bass_kernel_spmd

    is_first = int(np.asarray(inputs["is_first"]))
    num_iters = 3 if is_first else 2
    consts, general_bias = _prep_host(inputs)

    key = (num_iters, general_bias)
    if key not in _CACHE:
        _CACHE[key] = _build(num_iters, general_bias)
    nc = _CACHE[key]

    bf = ml_dtypes.bfloat16
    x = inputs["image_features"].astype(np.float32)
    xb = x.astype(bf)                                             # [64, 4096, 512]
    xTb = np.ascontiguousarray(x.transpose(0, 2, 1)).astype(bf)   # [64, 512, 4096]
    slots = inputs["slots"].astype(np.float32)                    # [64, 16, 128]

    in_maps = []
    for c in range(NCORES):
        sl = slice(c * BEX, (c + 1) * BEX)
        m = dict(consts)
        m["x"] = xb[sl]
        m["xT"] = xTb[sl]
        m["slots0"] = slots[sl].reshape(128, SLOT_DIM)
        in_maps.append(m)

    res = run_bass_kernel_spmd(nc, in_maps, list(range(NCORES)))
    out = np.stack([res.results[c]["out"] for c in range(NCORES)])  # [8, 128, 128]
    return out.reshape(B, NUM_SLOTS, SLOT_DIM)


if __name__ == "__main__":
    import reference
    inp = reference.setup_inputs()
    inp = {k: np.asarray(v) for k, v in inp.items()}
    got = kernel(**inp)
    exp = np.asarray(reference.reference(**reference.setup_inputs()))
    err = np.linalg.norm(got - exp) / np.linalg.norm(exp)
    print("Relative error:", err)

